# revision 4
# baseline (speedup 1.0000x reference)
"""MCANet forward on 8 Trainium2 NeuronCores (Bass/Tile), data-parallel over batch.

Per core: 4 samples (LD=512, LP=4096, H=128). Cost-model-driven design:
  - PE computes aff = p_feat @ d_feat.T once per sample, orientation [m, l]:
    32 matmul tiles [128m, 512l] into PSUM f32 (bf16 inputs), grouped in
    3-tile PSUM slots, double-buffered. A PE warmup burst of identity
    transposes holds the clock at full p-state before the first real matmul.
  - Act (scalar engine) converts each PSUM group to SBUF fp16 with func=Exp:
    since max(exp(x)) == exp(max(x)), all downstream max-reductions run in
    exp space and the softmax numerators come out of the maxes directly.
  - DVE computes per-tile colmax via tensor_scalar(op1=max, accum_out) in
    4x perf mode (0.26 ns/elem), and the rowmax as a chunk-pipelined
    pairfold tree with tensor_tensor(max) in 2x mode: each 8-tile chunk is
    folded to one [128, 512] row as soon as it is converted.
  - Final max-over-partition for the rowmax: PE transposes the folded row
    (fp16 -> PSUM) and one DVE tensor_reduce yields rmax [128l, 4].
  - Softmax denominators via ones-matmul partition sums; reciprocal on DVE;
    broadcast back to 128 partitions with tiny matmuls; fp16 weights.
  - Attention pooling: 36 accumulating 1-column matmuls (near-free on PE:
    cost scales with output free size). Per-sample 2-layer MLP.
Emission is software-pipelined: each sample's reduction/pooling tail is
generator-interleaved into the next sample's convert stream so no engine
waits on another sample's chain. gpsimd is used only for make_identity
(generic vector ops are not encodable on the Pool engine).
"""

import os
import sys

sys.path.insert(0, "/opt/trn_rl_repo")
_HERE = os.path.dirname(os.path.abspath(__file__))
if _HERE not in sys.path:
    sys.path.insert(0, _HERE)

import numpy as np
import ml_dtypes

import concourse.bass as bass
import concourse.tile as tile
from concourse import mybir
from concourse.bass_utils import run_bass_kernel_spmd
from concourse.masks import make_identity

F32 = mybir.dt.float32
BF16 = mybir.dt.bfloat16
FP16 = mybir.dt.float16
AF = mybir.ActivationFunctionType
OP = mybir.AluOpType
AX = mybir.AxisListType
NCORES = 8
B, LD, LP, H = 32, 512, 4096, 128
SPC = B // NCORES  # samples per core
NLT = LD // 128    # 4  l-tiles
NMT = LP // 128    # 32 m-tiles

_MAX_WAITS = int(os.environ.get("KERNEL_MAX_WAITS", "1"))

# Convert-group sizes along the 32 m-tiles: Act converts groups of 3 tiles
# (3-bank PSUM slots, double-buffered), last group is a pair. Sample 0
# starts with small groups so the pipeline fills faster.
GROUPS = [3] * 10 + [2]
GROUPS_S0 = [1, 2] + [3] * 9 + [2]
assert sum(GROUPS) == NMT and sum(GROUPS_S0) == NMT


def _split_excess_waits(nc, max_waits=_MAX_WAITS):
    """This walrus build rejects instructions carrying more than ~2 sync
    waits ("Too many sync wait commands"). Hoist excess waits onto injected
    same-engine NOPs placed immediately before the instruction."""
    import bass_rust

    cnt = 0
    for bb in nc.main_func.blocks:
        old = list(bb.instructions)
        need = any(
            ins.sync_info is not None and len(ins.sync_info.on_wait) > max_waits
            for ins in old
        )
        if not need:
            continue
        new = []
        for ins in old:
            si = ins.sync_info
            waits = list(si.on_wait) if si is not None else []
            if len(waits) > max_waits:
                chunks = [
                    waits[i : i + max_waits] for i in range(0, len(waits), max_waits)
                ]
                for ch in chunks[:-1]:
                    nop = mybir.InstNoOp(name=f"wsplit_{cnt}", ins=[], outs=[])
                    cnt += 1
                    nop.engine = ins.engine
                    nop.sync_info = bass_rust.SyncInfo(on_wait=ch, on_update=[])
                    new.append(nop)
                ins.sync_info = bass_rust.SyncInfo(
                    on_wait=chunks[-1], on_update=si.on_update
                )
            new.append(ins)
        bb.instructions = new
    return cnt


class _SplitDrainTileContext(tile.TileContext):
    def _drain_and_barrier(self, tick_clock, wait_clock):
        super()._drain_and_barrier(tick_clock, wait_clock)
        n = _split_excess_waits(self.nc)
        print(f"[kernel] split {n} excess-wait chunks onto nops")


def _build_nc():
    nc = bass.Bass()
    pfT_d = nc.declare_dram_parameter("pfT", [SPC, 128, LP], BF16, isOutput=False)
    dfT_d = nc.declare_dram_parameter("dfT", [SPC, 128, LD], BF16, isOutput=False)
    pfn_d = nc.declare_dram_parameter("pfn", [SPC, 128, NMT, 128], FP16, isOutput=False)
    dfn_d = nc.declare_dram_parameter("dfn", [SPC, 128, NLT, 128], FP16, isOutput=False)
    wp_d = nc.declare_dram_parameter("wpack", [128, 131], F32, isOutput=False)
    out_d = nc.declare_dram_parameter("out", [1, SPC], F32, isOutput=True)

    with _SplitDrainTileContext(nc) as tc:
        with (
            tc.tile_pool(name="singles", bufs=1) as singles,
            tc.tile_pool(name="feat", bufs=2) as feat,
            tc.tile_pool(name="aff", bufs=2) as affp,
            tc.tile_pool(name="stats", bufs=2) as stats,
            tc.tile_pool(name="pa", bufs=2, space="PSUM") as pa,
            tc.tile_pool(name="pm", bufs=2, space="PSUM") as pm,
        ):
            # ---- one-time constants (identity first: PE warmup needs it) ----
            ident = singles.tile([128, 128], FP16)
            make_identity(nc, ident)
            pwarm0 = pm.tile([128, 512], F32, tag="pm")
            pwarm0_16 = pwarm0.bitcast(FP16)
            for _ in range(22):
                nc.tensor.transpose(pwarm0_16[:, 0:128], ident[:], ident[:])
            onescol = singles.tile([128, 1], F32)
            nc.vector.memset(onescol, 1.0)
            onesrow = singles.tile([1, 128], F32)
            nc.vector.memset(onesrow, 1.0)
            wpack = singles.tile([128, 131], F32)
            w1_sb = wpack[:, 0:128].rearrange("p (c o) -> p c o", c=2)
            b1_sb = wpack[:64, 128:129]
            w2_sb = wpack[:64, 129:130]
            b2_sb = wpack[:1, 130:131]
            dvall = singles.tile([128, 2, SPC], F32)

            state = {}
            tstate = {}


            def emit_heavy(s, inject=None):
                dfT = feat.tile([128, LD], BF16, tag="dfT")
                nc.sync.dma_start(out=dfT, in_=dfT_d[s])
                pfT = feat.tile([128, LP], BF16, tag="pfT")
                # graded chunks so the first matmuls start early
                c0 = 0
                for cw in (512, 512, 1024, 2048):
                    nc.sync.dma_start(
                        out=pfT[:, c0 : c0 + cw], in_=pfT_d[s][:, c0 : c0 + cw]
                    )
                    c0 += cw
                pfn = feat.tile([128, NMT, 128], FP16, tag="pfn")
                nc.sync.dma_start(out=pfn, in_=pfn_d[s])
                dfn = feat.tile([128, NLT, 128], FP16, tag="dfn")
                nc.sync.dma_start(out=dfn, in_=dfn_d[s])
                if s == 0:
                    nc.sync.dma_start(out=wpack, in_=wp_d[:])

                arr = affp.tile([128, NMT, 512], FP16, tag="arr")
                # mx holds [rmax | cmax] so one exp covers both later
                mx = stats.tile([128, NLT + NMT], F32, tag="mx")
                scr = stats.tile([128, 512], FP16, tag="scr")
                cmax = mx[:, NLT : NLT + NMT]

                def cmax_col(j):
                    return cmax[:, j : j + 1]

                def convert_group(g0, gn):
                    psg = pa.tile([128, 3, 512], F32, tag="pa", name="psg")
                    for k in range(gn):
                        nc.tensor.matmul(
                            psg[:, k, :],
                            lhsT=pfT[:, (g0 + k) * 128 : (g0 + k + 1) * 128],
                            rhs=dfT[:],
                            start=True,
                            stop=True,
                        )
                    # exp-convert: max(exp) == exp(max), so downstream maxes
                    # work directly in exp space and no separate exp is needed
                    nc.scalar.activation(
                        arr[:, g0 : g0 + gn, :], psg[:, 0:gn, :], AF.Exp
                    )
                    for k in range(gn):
                        nc.vector.tensor_scalar(
                            out=scr[:],
                            in0=arr[:, g0 + k, :],
                            scalar1=1.0,
                            scalar2=None,
                            op0=OP.mult,
                            op1=OP.max,
                            accum_out=cmax_col(g0 + k),
                        )

                # chunked fold: after each 8 converted tiles, DVE pairfolds
                # the chunk so only a shallow ladder trails the last convert.
                f4 = affp.tile([128, 4, 4, 512], FP16, tag="f4", bufs=1)
                f2 = affp.tile([128, 4, 2, 512], FP16, tag="f2", bufs=1)
                fr = affp.tile([128, 4, 512], FP16, tag="fr", bufs=1)
                racc = affp.tile([128, 512], FP16, tag="racc")

                def fold_chunk(r):
                    t0 = r * 8
                    nc.vector.tensor_tensor(
                        out=f4[:, r, :, :],
                        in0=arr[:, t0 : t0 + 8 : 2, :],
                        in1=arr[:, t0 + 1 : t0 + 8 : 2, :],
                        op=OP.max,
                    )
                    nc.vector.tensor_tensor(
                        out=f2[:, r, :, :],
                        in0=f4[:, r, 0:4:2, :],
                        in1=f4[:, r, 1:4:2, :],
                        op=OP.max,
                    )
                    nc.vector.tensor_tensor(
                        out=fr[:, r, :], in0=f2[:, r, 0, :], in1=f2[:, r, 1, :],
                        op=OP.max,
                    )

                j = 0
                fold_done = 0
                for gn in (GROUPS_S0 if s == 0 else GROUPS):
                    convert_group(j, gn)
                    j += gn
                    while fold_done < j // 8:
                        fold_chunk(fold_done)
                        fold_done += 1
                    if inject is not None:
                        inject(j)

                f2b = affp.tile([128, 2, 512], FP16, tag="f2b", bufs=1)
                nc.vector.tensor_tensor(
                    out=f2b[:], in0=fr[:, 0:2, :], in1=fr[:, 2:4, :], op=OP.max
                )
                nc.vector.tensor_tensor(
                    out=racc[:], in0=f2b[:, 0, :], in1=f2b[:, 1, :], op=OP.max
                )
                state[s] = (pfn, dfn, mx, racc, scr)

            outv = singles.tile([1, SPC], F32)

            def tail_a(s):
                """Boundary block: transposes + rowmax reduce (exp space)."""
                pfn, dfn, mx, racc, scr = state.pop(s)
                # pm bank layout (f32 cols): fp16 transposes occupy f32 cols
                # 0:256; denominators 256:292; pooling 384:386; MLP hidden
                # 400:401, out 496:497.
                pmix = pm.tile([128, 512], F32, tag="pm")
                pmix16 = pmix.bitcast(FP16)
                for c in range(NLT):
                    nc.tensor.transpose(
                        pmix16[:, c * 128 : (c + 1) * 128],
                        racc[:, c * 128 : (c + 1) * 128],
                        ident[:],
                    )
                nc.vector.tensor_reduce(
                    out=mx[:, 0:NLT],
                    in_=pmix16[:, 0:512].rearrange("p (c n) -> p c n", c=NLT),
                    axis=AX.X,
                    op=OP.max,
                )
                tstate[s] = (pfn, dfn, mx, pmix)

            def tail_b(s):
                """Rest of the tail as a generator: yields between pieces so
                the caller can interleave the next sample's converts. The
                p-side (colmax) path runs first since it does not depend on
                the rowmax fold ladder."""
                pfn, dfn, mx, pmix = tstate.pop(s)
                erm = mx[:, 0:NLT]
                ecm = mx[:, NLT : NLT + NMT]
                # p-side: denominator, reciprocal, weights, pooling
                nc.tensor.matmul(
                    pmix[:1, 260:292], lhsT=onescol[:], rhs=ecm[:], start=True,
                    stop=True,
                )
                yield
                dsum = stats.tile([1, 2], F32, tag="dsum")
                nc.vector.reduce_sum(dsum[:1, 1:2], pmix[:1, 260:292], axis=AX.X)
                rec = stats.tile([1, 2], F32, tag="rec")
                nc.vector.reciprocal(rec[:1, 1:2], dsum[:1, 1:2])
                yield
                nc.tensor.matmul(
                    pmix[:, 353:354], lhsT=onesrow[:], rhs=rec[:1, 1:2],
                    start=True, stop=True,
                )
                yield
                rsc = stats.tile([128, 2], F32, tag="rsc")
                nc.scalar.activation(rsc[:, 1:2], pmix[:, 353:354], AF.Copy)
                wp = stats.tile([128, NMT], FP16, tag="wp")
                nc.vector.tensor_scalar(
                    out=wp[:], in0=ecm[:], scalar1=rsc[:, 1:2], scalar2=None,
                    op0=OP.mult,
                )
                yield
                for m in range(NMT):
                    nc.tensor.matmul(
                        pmix[:, 385:386], lhsT=pfn[:, m, :], rhs=wp[:, m : m + 1],
                        start=(m == 0), stop=(m == NMT - 1),
                    )
                yield
                # d-side: waits on the rowmax ladder result (erm)
                nc.tensor.matmul(
                    pmix[:1, 256:260], lhsT=onescol[:], rhs=erm[:], start=True,
                    stop=True,
                )
                yield
                nc.vector.reduce_sum(dsum[:1, 0:1], pmix[:1, 256:260], axis=AX.X)
                nc.vector.reciprocal(rec[:1, 0:1], dsum[:1, 0:1])
                yield
                nc.tensor.matmul(
                    pmix[:, 352:353], lhsT=onesrow[:], rhs=rec[:1, 0:1],
                    start=True, stop=True,
                )
                yield
                nc.scalar.activation(rsc[:, 0:1], pmix[:, 352:353], AF.Copy)
                wd = stats.tile([128, NLT], FP16, tag="wd")
                nc.vector.tensor_scalar(
                    out=wd[:], in0=erm[:], scalar1=rsc[:, 0:1], scalar2=None,
                    op0=OP.mult,
                )
                yield
                for t in range(NLT):
                    nc.tensor.matmul(
                        pmix[:, 384:385], lhsT=dfn[:, t, :], rhs=wd[:, t : t + 1],
                        start=(t == 0), stop=(t == NLT - 1),
                    )
                yield
                nc.scalar.activation(dvall[:, :, s], pmix[:, 384:386], AF.Copy)
                yield
                # per-sample MLP: relu([d;p] @ W1 + b1) @ W2 + b2
                nc.tensor.matmul(
                    pmix[:64, 400:401], lhsT=w1_sb[:, 0, :],
                    rhs=dvall[:, 0, s : s + 1], start=True, stop=False,
                )
                nc.tensor.matmul(
                    pmix[:64, 400:401], lhsT=w1_sb[:, 1, :],
                    rhs=dvall[:, 1, s : s + 1], start=False, stop=True,
                )
                yield
                hb = stats.tile([64, 1], F32, tag="hb")
                nc.scalar.activation(
                    hb[:], pmix[:64, 400:401], AF.Relu, bias=b1_sb[:, 0:1]
                )
                nc.tensor.matmul(
                    pmix[:1, 496:497], lhsT=w2_sb[:], rhs=hb[:], start=True, stop=True
                )
                yield
                nc.scalar.activation(
                    outv[:, s : s + 1], pmix[:1, 496:497], AF.Identity,
                    bias=b2_sb[:, 0:1],
                )
                nc.sync.dma_start(out=out_d[:, s : s + 1], in_=outv[:, s : s + 1])

            for step in range(SPC):
                prev = step - 1
                if prev >= 0:
                    tail_a(prev)
                    tb = tail_b(prev)

                    def inject(j, tb=tb):
                        if j % 4 == 0:
                            try:
                                next(tb)
                            except StopIteration:
                                pass

                    emit_heavy(step, inject=inject)
                    for _ in tb:
                        pass
                else:
                    emit_heavy(step)
            tail_a(SPC - 1)
            for _ in tail_b(SPC - 1):
                pass
    return nc


_NC_CACHE = None


def kernel(drug_ids, prot_ids, drug_emb, prot_emb, W1, b1, W2, b2):
    global _NC_CACHE
    drug_ids = np.asarray(drug_ids)
    prot_ids = np.asarray(prot_ids)
    drug_emb = np.asarray(drug_emb, dtype=np.float32)
    prot_emb = np.asarray(prot_emb, dtype=np.float32)
    W1 = np.asarray(W1, dtype=np.float32)
    b1 = np.asarray(b1, dtype=np.float32)
    W2 = np.asarray(W2, dtype=np.float32)
    b2 = np.asarray(b2, dtype=np.float32)

    # host-side gather of the small tables into matmul-friendly layouts
    d_feat = drug_emb[drug_ids]  # [B, LD, H]
    p_feat = prot_emb[prot_ids]  # [B, LP, H]
    dfT = np.ascontiguousarray(d_feat.transpose(0, 2, 1)).astype(ml_dtypes.bfloat16)
    pfT = np.ascontiguousarray(p_feat.transpose(0, 2, 1)).astype(ml_dtypes.bfloat16)
    dfn = np.ascontiguousarray(
        d_feat.reshape(B, NLT, 128, H).transpose(0, 2, 1, 3)
    ).astype(np.float16)  # [B, 128, NLT, H]
    pfn = np.ascontiguousarray(
        p_feat.reshape(B, NMT, 128, H).transpose(0, 2, 1, 3)
    ).astype(np.float16)  # [B, 128, NMT, H]

    wpk = np.zeros((128, 131), dtype=np.float32)
    wpk[:, 0:128] = W1.reshape(2, 128, 64).transpose(1, 0, 2).reshape(128, 128)
    wpk[:64, 128] = b1
    wpk[:64, 129] = W2[:, 0]
    wpk[0, 130] = b2[0]

    if _NC_CACHE is None:
        _NC_CACHE = _build_nc()
    nc = _NC_CACHE

    in_maps = []
    for c in range(NCORES):
        sl = slice(c * SPC, (c + 1) * SPC)
        in_maps.append(
            {
                "pfT": pfT[sl],
                "dfT": dfT[sl],
                "pfn": pfn[sl],
                "dfn": dfn[sl],
                "wpack": wpk,
            }
        )

    trace = bool(os.environ.get("KERNEL_TRACE"))
    res = run_bass_kernel_spmd(nc, in_maps, list(range(NCORES)), trace=trace)
    kernel.last_result = res
    out = np.concatenate(
        [res.results[c]["out"].reshape(SPC, 1) for c in range(NCORES)], axis=0
    )
    return out.astype(np.float32)


kernel.last_result = None
